# revision 3
# baseline (speedup 1.0000x reference)
"""Trainium2 Bass kernel for nn_DART_Net (gnn_message_passing).

Math (per molecule b, node n):
  hi = mlp2_i(ai) * mask(ai)                 [B,N,128]
  h{j,k,l} = mlp2_t(a_t) * mask(a_t)        [B,N,M,128] -> sum over M
  atm = hi + sum_j + sum_k + sum_l
  out = (celu-chain(atm) @ W4.T + b4) * mask(ai)
with mlp2(x) = celu(celu(x@W1.T+b1)@W2.T+b2), celu alpha=0.1.

Device strategy (pure data parallel over B, 8 molecules/core):
  - features on partitions, message rows on the free axis, m-major within
    each 1024-column group (host pre-permutes) so the M-sum runs as PE
    "fan-in" matmuls with a repeated (step-0) output access pattern.
  - all matmuls at 1 cycle/column: float32r for the contraction-4 L1 and
    rank-1 bias matmuls, fp16 for L2 / fan-in (weights shipped fp16).
  - celu via the shifted identity  celu(v)+a = max(z', min(t, a)),
    z' = v+a (delivered in PSUM by the matmul: ones-row for L1, rank-1
    ones matmul for later layers), t = a*e^{(z'-a)/a}:
      route A (ACT+DVE):  t = ACT Exp;  e = DVE stc (t min a) max z'
      route E (ACT+GPS+DVE, tensors j,k layer 1):
          y  = ACT Identity drain of z' (PSUM->SBUF fp16)
          ib = GPS tensor_scalar  round(A*y + B) -> int32   (Schraudolph)
          q  = GPS tensor_scalar  min(f32view(ib), a) -> fp16
          e  = DVE tensor_tensor  max(q, y)  (2-byte operands, 2x mode)
    This spreads the elementwise work across ACT/DVE/GPS.
  - exact-zero-sum padding rows fixed up via a host-computed per-node
    correction entering the trunk matmul; ai mask applied on device.
"""

import sys
import numpy as np
from contextlib import ExitStack

for _p in ("/opt/trn_rl_repo", "/root/.axon_site/_ro/trn_rl_repo"):
    if _p not in sys.path:
        sys.path.append(_p)

ALPHA = 0.1
INV_ALPHA = 1.0 / ALPHA
# Exp(z'*INV_ALPHA + EXPB) == alpha * e^{(z'-alpha)/alpha}
EXPB = float(np.log(np.float32(ALPHA)) - 1.0)
# Schraudolph: float-bits(t) ~= SCHR_A*z' + SCHR_B for t = alpha*e^{(z'-a)/a}
SCHR_A = float(2.0**23 * np.log2(np.e) / ALPHA)
SCHR_B = float(2.0**23 * (127 + np.log2(ALPHA) - np.log2(np.e)) - 366000.0)

B, N, M = 64, 128, 64
NCORES = 8
BL = B // NCORES          # molecules per core
LH, LO = 128, 128
C1, C2, C3 = 64, 32, 16

E4_TENSORS = ("j", "k")   # L1 celu via GPS-Schraudolph route for these

_PROGRAM_CACHE = {}


# --------------------------------------------------------------------------
# device program
# --------------------------------------------------------------------------

def _build_program(nmol=BL, debug=False):
    import concourse.bass as bass
    import concourse.tile as tile
    from concourse import bacc, mybir

    f32 = mybir.dt.float32
    f32r = mybir.dt.float32r
    fp16 = mybir.dt.float16
    i32 = mybir.dt.int32
    Alu = mybir.AluOpType
    Act = mybir.ActivationFunctionType

    nodes = nmol * N                  # 1024 nodes per core
    rmsg = nodes * M                  # 65536 message rows per tensor
    PH = 512                          # nodes per trunk phase (1 PSUM bank)
    nphase = nodes // PH
    G = 1024                          # message columns per group
    npg = G // M                      # nodes per group (16)
    ngrp = PH // npg                  # groups per phase (32)

    nc = bacc.Bacc("TRN2", target_bir_lowering=False, debug=debug)

    x_dram = {t: nc.dram_tensor(f"x{t}", [4, rmsg], f32r, kind="ExternalInput")
              for t in "jkl"}
    xi_dram = nc.dram_tensor("xi", [4, nodes], f32r, kind="ExternalInput")
    corr_dram = nc.dram_tensor("corr", [LO, nodes], f32r, kind="ExternalInput")
    mi_dram = nc.dram_tensor("mi", [1, nodes], f32, kind="ExternalInput")

    w_specs = {}
    for t in "jkli":
        w_specs[f"w1{t}"] = ([4, LH], f32r)     # [W1.T ; b1+alpha]
        w_specs[f"w2{t}"] = ([LH, LO], fp16)    # W2.T
        w_specs[f"bb2{t}"] = ([1, LO], f32r)    # b2 + a - a*rowsum(W2)
    w_specs.update(
        wc1h=([LO, C1], fp16), wc1f=([LO, C1], f32r), bbtr=([1, C1], f32r),
        wc2=([C1, C2], fp16), bbc2=([1, C2], f32r),
        wc3=([C2, C3], fp16), bbc3=([1, C3], f32r),
        wc4=([C3, 1], fp16), bc4=([1, 1], f32))
    w_dram = {k: nc.dram_tensor(k, shp, dt, kind="ExternalInput")
              for k, (shp, dt) in w_specs.items()}
    out_dram = nc.dram_tensor("out", [1, nodes], f32, kind="ExternalOutput")

    with ExitStack() as ctx:
        tc = ctx.enter_context(tile.TileContext(nc))

        wpool = ctx.enter_context(tc.tile_pool(name="w", bufs=1))
        xpool = ctx.enter_context(tc.tile_pool(name="x", bufs=4))
        za_pool = ctx.enter_context(tc.tile_pool(name="za", bufs=2, space="PSUM"))
        zb_pool = ctx.enter_context(tc.tile_pool(name="zb", bufs=1, space="PSUM"))
        sm_pool = ctx.enter_context(tc.tile_pool(name="sm", bufs=1, space="PSUM"))
        tr_pool = ctx.enter_context(tc.tile_pool(name="tr", bufs=1, space="PSUM"))
        t1_pool = ctx.enter_context(tc.tile_pool(name="t1", bufs=3))
        ii_pool = ctx.enter_context(tc.tile_pool(name="ii", bufs=2))
        qq_pool = ctx.enter_context(tc.tile_pool(name="qq", bufs=2))
        e1_pool = ctx.enter_context(tc.tile_pool(name="e1", bufs=2))
        e2_pool = ctx.enter_context(tc.tile_pool(name="e2", bufs=2))
        small = ctx.enter_context(tc.tile_pool(name="small", bufs=2))

        wsb = {}
        for k, (shp, dt) in w_specs.items():
            wt = wpool.tile(shp, dt, tag=f"w_{k}")
            nc.sync.dma_start(wt[:], w_dram[k][:])
            wsb[k] = wt
        corr_sb = wpool.tile([LO, nodes], f32r, tag="corr")
        nc.sync.dma_start(corr_sb[:], corr_dram[:])
        mi_sb = wpool.tile([1, nodes], f32, tag="mi")
        nc.sync.dma_start(mi_sb[:], mi_dram[:])
        xi_sb = wpool.tile([4, nodes], f32r, tag="xi")
        nc.sync.dma_start(xi_sb[:], xi_dram[:])
        ones_sb = wpool.tile([1, 512], f32r, tag="ones")
        nc.vector.memset(ones_sb[:], 1.0)
        expb_sb = wpool.tile([128, 1], f32, tag="expb")
        nc.vector.memset(expb_sb[:], EXPB)

        def celu_a2(zp, out, tag="t1"):
            """out(fp16) = max(zp, min(t, a)); zp(PSUM) holds z+b+alpha."""
            P = zp.shape[0]
            tt = t1_pool.tile([P, zp.shape[-1]], fp16, tag=tag)
            nc.scalar.activation(tt[:], zp, Act.Exp, bias=expb_sb[:P, :],
                                 scale=INV_ALPHA)
            nc.vector.scalar_tensor_tensor(out, tt[:], ALPHA, zp,
                                           Alu.min, Alu.max)

        def celu_e4(zp, out):
            """Same result via ACT drain + GPS Schraudolph + DVE max."""
            P, W = zp.shape[0], zp.shape[-1]
            y = t1_pool.tile([P, W], fp16, tag="t1")
            nc.scalar.activation(y[:], zp, Act.Identity, bias=0.0, scale=1.0)
            ib = ii_pool.tile([P, W], i32, tag="ii")
            nc.gpsimd.tensor_scalar(ib[:], y[:], SCHR_A, SCHR_B,
                                    Alu.mult, Alu.add)
            q = qq_pool.tile([P, W], fp16, tag="qq")
            nc.gpsimd.tensor_scalar(q[:], ib[:].bitcast(f32), ALPHA, None,
                                    Alu.min, Alu.bypass)
            nc.vector.tensor_tensor(out, q[:], y[:], op=Alu.max)

        def bias_mm(zp, key, width):
            nc.tensor.matmul(zp, wsb[key][:], ones_sb[:, :width],
                             start=False, stop=False, skip_group_check=True)

        for p in range(nphase):
            nsl = slice(p * PH, (p + 1) * PH)

            # ---- ai path + trunk init ----
            zi = sm_pool.tile([LH, PH], f32, tag="sm")
            nc.tensor.matmul(zi[:], wsb["w1i"][:], xi_sb[:, nsl],
                             start=True, stop=True, skip_group_check=True)
            e1i = small.tile([LH, PH], fp16, tag="e1i")
            celu_a2(zi[:], e1i[:])
            zi2 = sm_pool.tile([LO, PH], f32, tag="sm")
            nc.tensor.matmul(zi2[:], wsb["w2i"][:], e1i[:], start=True,
                             stop=False, skip_group_check=True)
            bias_mm(zi2[:], "bb2i", PH)
            e2i = small.tile([LO, PH], fp16, tag="e2i")
            celu_a2(zi2[:], e2i[:])

            trunk = tr_pool.tile([C1, PH], f32, tag="trunk")
            nc.tensor.matmul(trunk[:], wsb["wc1h"][:], e2i[:],
                             start=True, stop=False, skip_group_check=True)
            nc.tensor.matmul(trunk[:], wsb["wc1f"][:], corr_sb[:, nsl],
                             start=False, stop=False, skip_group_check=True)
            bias_mm(trunk[:], "bbtr", PH)

            # ---- message streams; column order within a group is m-major ----
            for g in range(ngrp):
                for t in "jkl":
                    off = (p * ngrp + g) * G
                    xg = xpool.tile([4, G], f32r, tag="xg")
                    nc.sync.dma_start(xg[:], x_dram[t][:, off:off + G])

                    za = za_pool.tile([LH, G], f32, tag="za")
                    for h in range(2):
                        cs = slice(h * 512, (h + 1) * 512)
                        nc.tensor.matmul(za[:, cs], wsb[f"w1{t}"][:], xg[:, cs],
                                         start=True, stop=True)
                    e1t = e1_pool.tile([LH, G], fp16, tag="e1")
                    if t in E4_TENSORS:
                        celu_e4(za[:], e1t[:])
                    else:
                        celu_a2(za[:], e1t[:])

                    zb = zb_pool.tile([LO, G], f32, tag="zb")
                    for h in range(2):
                        cs = slice(h * 512, (h + 1) * 512)
                        nc.tensor.matmul(zb[:, cs], wsb[f"w2{t}"][:],
                                         e1t[:, cs], start=True, stop=False,
                                         skip_group_check=True)
                        bias_mm(zb[:, cs], f"bb2{t}", 512)
                    e2t = e2_pool.tile([LO, G], fp16, tag="e2")
                    celu_a2(zb[:], e2t[:], tag="t2")

                    # m-sum via PE fan-in (m-major: addr = m*npg + n)
                    tv = trunk[:, g * npg:(g + 1) * npg]
                    fan_ap = bass.AP(tensor=tv.tensor, offset=tv.offset,
                                     ap=[list(tv.ap[0]), [0, M // 2],
                                         list(tv.ap[-1])])
                    for h in range(2):
                        cs = slice(h * 512, (h + 1) * 512)
                        nc.tensor.matmul(fan_ap, wsb["wc1h"][:], e2t[:, cs],
                                         start=False, stop=False,
                                         skip_group_check=True)

            # ---- trunk chain ----
            ec1 = small.tile([C1, PH], fp16, tag="ec1")
            celu_a2(trunk[:], ec1[:])
            z2c = sm_pool.tile([C2, PH], f32, tag="sm")
            nc.tensor.matmul(z2c[:], wsb["wc2"][:], ec1[:], start=True,
                             stop=False, skip_group_check=True)
            bias_mm(z2c[:], "bbc2", PH)
            ec2 = small.tile([C2, PH], fp16, tag="ec2")
            celu_a2(z2c[:], ec2[:])
            z3c = sm_pool.tile([C3, PH], f32, tag="sm")
            nc.tensor.matmul(z3c[:], wsb["wc3"][:], ec2[:], start=True,
                             stop=False, skip_group_check=True)
            bias_mm(z3c[:], "bbc3", PH)
            ec3 = small.tile([C3, PH], fp16, tag="ec3")
            celu_a2(z3c[:], ec3[:])
            z4c = sm_pool.tile([1, PH], f32, tag="sm")
            nc.tensor.matmul(z4c[:], wsb["wc4"][:], ec3[:], start=True,
                             stop=True, skip_group_check=True)
            o = small.tile([1, PH], f32, tag="o")
            nc.scalar.activation(o[:], z4c[:], Act.Identity,
                                 bias=wsb["bc4"][:], scale=1.0)
            om = small.tile([1, PH], f32, tag="om")
            nc.vector.tensor_mul(om[:], o[:], mi_sb[:, nsl])
            nc.sync.dma_start(out_dram[:, nsl], om[:])

    nc.compile()
    return nc


# --------------------------------------------------------------------------
# host side
# --------------------------------------------------------------------------

def _celu_np(x):
    x = x.astype(np.float32)
    return (np.maximum(x, 0.0)
            + np.minimum(0.0, np.float32(ALPHA)
                         * np.expm1(x * np.float32(INV_ALPHA)))).astype(np.float32)


def _msg_layout(a):
    """[nmol,N,M,3] -> [4, rmsg] f32 with m-major 1024-col groups + ones row."""
    nodes = a.shape[0] * a.shape[1]
    npg = 1024 // M
    # group-major, m-major within group: col = grp*1024 + m*npg + n_local
    g = a.reshape(nodes // npg, npg, M, 3).transpose(0, 2, 1, 3)
    flat = np.ascontiguousarray(g, dtype=np.float32).reshape(-1, 3)
    out = np.empty((4, flat.shape[0]), np.float32)
    out[:3] = flat.T
    out[3] = 1.0
    return out


def _with_ones(flat):
    out = np.empty((4, flat.shape[0]), np.float32)
    out[:3] = flat.T
    out[3] = 1.0
    return out


def _prep_core(inputs, c, nmol=BL):
    s = slice(c * nmol, (c + 1) * nmol)
    nodes = nmol * N
    a = np.float32(ALPHA)
    d = {}
    for t, key in (("j", "aj"), ("k", "ak"), ("l", "al")):
        d[f"x{t}"] = _msg_layout(np.asarray(inputs[key][s], np.float32))
    ai = np.ascontiguousarray(inputs["ai"][s], dtype=np.float32).reshape(-1, 3)
    d["xi"] = _with_ones(ai)
    mi = ((ai[:, 0] + ai[:, 1]) + ai[:, 2]) != 0
    d["mi"] = mi.astype(np.float32)[None, :]

    corr = np.zeros((nodes, LO), np.float32)
    for key, wn in (("aj", "j"), ("ak", "k"), ("al", "l")):
        flat = np.ascontiguousarray(inputs[key][s], dtype=np.float32).reshape(-1, 3)
        ssum = (flat[:, 0] + flat[:, 1]) + flat[:, 2]
        idx = np.nonzero(ssum == 0)[0]
        if idx.size:
            W1 = inputs[f"W{wn}1"].astype(np.float32)
            b1 = inputs[f"b{wn}1"].astype(np.float32)
            W2 = inputs[f"W{wn}2"].astype(np.float32)
            b2 = inputs[f"b{wn}2"].astype(np.float32)
            h1 = _celu_np(flat[idx] @ W1.T + b1)
            h2 = _celu_np(h1 @ W2.T + b2)
            np.subtract.at(corr, idx // M, h2)
    d["corr"] = np.ascontiguousarray(corr.T)

    def ct(x):
        return np.ascontiguousarray(x, dtype=np.float32)

    def ch(x):
        return np.ascontiguousarray(x, dtype=np.float16)

    for t, wn in (("j", "j"), ("k", "k"), ("l", "l"), ("i", "i")):
        W1 = inputs[f"W{wn}1"].astype(np.float32)
        b1 = inputs[f"b{wn}1"].astype(np.float32)
        W2 = inputs[f"W{wn}2"].astype(np.float32)
        b2 = inputs[f"b{wn}2"].astype(np.float32)
        d[f"w1{t}"] = ct(np.vstack([W1.T, (b1 + a)[None, :]]))
        d[f"w2{t}"] = ch(W2.T)
        # input to L2 is e1+alpha -> subtract a*rowsum(W2); then +b2+alpha
        d[f"bb2{t}"] = ct(b2 + a - a * W2.sum(axis=1))[None, :]

    W1c = inputs["W1"].astype(np.float32); b1c = inputs["b1"].astype(np.float32)
    W2c = inputs["W2"].astype(np.float32); b2c = inputs["b2"].astype(np.float32)
    W3c = inputs["W3"].astype(np.float32); b3c = inputs["b3"].astype(np.float32)
    W4c = inputs["W4"].astype(np.float32); b4c = inputs["b4"].astype(np.float32)
    d["wc1h"] = ch(W1c.T)
    d["wc1f"] = ct(W1c.T)
    # trunk input is the sum of 193 alpha-shifted activations (192 msgs + ai)
    d["bbtr"] = ct(b1c + a - (3 * M + 1) * a * W1c.sum(axis=1))[None, :]
    d["wc2"] = ch(W2c.T)
    d["bbc2"] = ct(b2c + a - a * W2c.sum(axis=1))[None, :]
    d["wc3"] = ch(W3c.T)
    d["bbc3"] = ct(b3c + a - a * W3c.sum(axis=1))[None, :]
    d["wc4"] = ch(W4c.T)
    d["bc4"] = ct(b4c - a * W4c.sum(axis=1))[:, None]
    return d


def _get_program(nmol=BL):
    key = (nmol, E4_TENSORS)
    if key not in _PROGRAM_CACHE:
        _PROGRAM_CACHE[key] = _build_program(nmol=nmol)
    return _PROGRAM_CACHE[key]


def run(inputs, trace=False, **kwargs):
    """Returns (full_output [B,N,1] f32, BassKernelResults)."""
    from concourse.bass_utils import run_bass_kernel_spmd
    inputs = {k: np.asarray(v) for k, v in inputs.items()}
    nc = _get_program()
    in_maps = [_prep_core(inputs, c) for c in range(NCORES)]
    res = run_bass_kernel_spmd(nc, in_maps, core_ids=list(range(NCORES)),
                               trace=trace, **kwargs)
    outs = [res.results[c]["out"].reshape(BL, N, 1) for c in range(NCORES)]
    return np.concatenate(outs, axis=0).astype(np.float32), res


def kernel(**inputs):
    out, _ = run(inputs)
    return out
